# revision 3
# baseline (speedup 1.0000x reference)
"""CausalWanS2V self-attention (sparse_attention) — 8-core head-sharded Trainium2 Bass kernel.

Layout strategy (per core c, heads 2c..2c+1):
  - Projections computed head-dim-major: qT/kT [hd=128, s] directly from
    lhsT = W^T tiles (host pre-transposed), rhs = x^T tiles (host pre-transposed).
  - qk RMSNorm is over the full 2048-dim q/k vector -> cross-core AllGather of
    per-core square-sum rows [2, 720] (tiny), then rsqrt on-device.
  - RoPE applied in T-layout via a pair-swap permutation matmul plus DVE muls
    against host-prepared (norm-weight x cos/sin, sign-folded) tables.
  - Attention in S^T layout: for each 128-row chunk of cache positions,
    S^T = kT_tile.T @ qT  (PE, f32r), E = exp(scale*S^T) (ACT, PSUM->SBUF),
    O^T += v_tile.T @ E (PE, accumulating PSUM), acc += E (DVE) for the
    softmax denominators; denominators = ones.T @ acc (PE, exact f32).
  - New tokens' k/v come from the on-device projections (cache rows
    local_start:local_start+720); old cache rows are streamed from host-packed,
    per-head transposed arrays.
  - O^T normalized by 1/denom (broadcast via ones-outer-product matmul), then
    per-head o-projection partials [720, 2048]; host sums the 8 partials + o_b.
"""
import sys

sys.path.insert(0, "/opt/trn_rl_repo")

import numpy as np
import concourse.bass as bass
import concourse.mybir as mybir
import concourse.tile as tile
from concourse import bacc
from concourse import bass_utils

f32 = mybir.dt.float32
f32r = mybir.dt.float32r
AF = mybir.ActivationFunctionType

# problem constants (hardcoded per contract)
SEQ = 720
SP = 768           # s padded to 512+256 so every matmul N-chunk is bank-legal & f32r full rate
DIM = 2048
NH = 16
HD = 128
CACHE = 11520
N_CORES = 8
HPC = NH // N_CORES        # heads per core = 2
HDC = HPC * HD             # 256 out dims per core
OLD = CACHE - SEQ          # 10800 old cache rows
SLAB = 2160                # kpos per DMA slab (5 slabs of 16*128+112)
NSLAB = OLD // SLAB
SM_SCALE = float(HD) ** -0.5
EPS = 1e-6
KT = DIM // 128            # 16 contraction tiles

# s-tiles of 720: 5 full 128s + one 80
S_TILES = [(i * 128, min(128, SEQ - i * 128)) for i in range((SEQ + 127) // 128)]
# tiles inside one 2160-row slab: 16 full 128s + one 112
SLAB_TILES = [(i * 128, min(128, SLAB - i * 128)) for i in range((SLAB + 127) // 128)]
NCHUNKS = ((0, 512), (512, 256))       # matmul N chunking of the padded s dim
NCHUNKS_REAL = ((0, 512), (512, 208))  # chunking of the real 720 (exact-f32 ops)


def _emit(nc, tc, d):
    """Emit the per-core program. d = dict of dram tensor handles."""
    ap = {k: v.ap() for k, v in d.items()}

    with tc.tile_pool(name="p0", bufs=1) as p0, \
         tc.tile_pool(name="dram", bufs=1, space="DRAM") as dpool, \
         tc.tile_pool(name="psS", bufs=3, space="PSUM") as psS, \
         tc.tile_pool(name="psO", bufs=1, space="PSUM") as psO:

        # ---- persistent tiles ----
        rq = [p0.tile([128, SP], f32r, tag=f"rq{h}", name=f"rq{h}") for h in range(HPC)]
        rk = [p0.tile([128, SP], f32r, tag=f"rk{h}", name=f"rk{h}") for h in range(HPC)]
        vs = [p0.tile([128, HDC], f32r, tag=f"vs{st}", name=f"vs{st}") for st in range(len(S_TILES))]
        OT = [p0.tile([128, SP], f32r, tag=f"ot{h}", name=f"ot{h}") for h in range(HPC)]
        acc = p0.tile([128, SP], f32, tag="acc")
        acc2 = p0.tile([128, SP], f32, tag="acc2")
        ones_col = p0.tile([128, 1], f32, tag="ones_col")
        ones_row = p0.tile([1, 128], f32, tag="ones_row")
        ones_row_r = p0.tile([1, SP], f32r, tag="ones_row_r")
        nc.sync.dma_start(ones_row_r[:], d["ones_r"].ap())
        recipf_q = p0.tile([1, SEQ], f32, tag="recipf_q")
        recipf_k = p0.tile([1, SEQ], f32, tag="recipf_k")
        recipf = (recipf_q, recipf_k)
        eps_t = p0.tile([1, 1], f32, tag="eps_t")
        prewarm = p0.tile([1, 1], f32, tag="prewarm")
        zeros_f32 = p0.tile([128, SP - SEQ], f32, tag="zeros_f32")
        nc.gpsimd.memset(eps_t[:], EPS)
        nc.gpsimd.memset(ones_col[:], 1.0)
        nc.gpsimd.memset(ones_row[:], 1.0)
        nc.gpsimd.memset(zeros_f32[:], 0.0)
        # pre-load the natural_log_exp table set while DMAs stream
        nc.scalar.activation(prewarm[:], eps_t[:], AF.Exp)
        for h in range(HPC):
            nc.vector.tensor_copy(rq[h][:, SEQ:SP], zeros_f32[:])
            nc.vector.tensor_copy(rk[h][:, SEQ:SP], zeros_f32[:])

        with tc.tile_pool(name="pa", bufs=1) as pa, \
             tc.tile_pool(name="work", bufs=1) as work:
            # ---- phase A loads ----
            # creation order controls SBUF placement: tiles that die early
            # (wq, wk, cos) sit at low addresses where the attention slab/E
            # pools will land; wv and xt (alive until the v projection) go
            # last so they don't block the slab DMAs
            wq = pa.tile([128, KT, HDC], f32r, tag="wq")
            wk = pa.tile([128, KT, HDC], f32r, tag="wk")
            cw = {}
            for nm in ("cosq", "sinq", "cosk", "sink"):
                cw[nm] = pa.tile([128, HPC * SEQ], f32, tag=nm, name=nm)
            wv = pa.tile([128, KT, HDC], f32r, tag="wv")
            xt = pa.tile([128, KT, SP], f32r, tag="xt")
            x_r = ap["xT"].rearrange("(g p) s -> p g s", p=128)
            w_rs = {n: ap[n].rearrange("(g p) n -> p g n", p=128)
                    for n in ("wqT", "wkT", "wvT")}
            # interleaved so the q-projection can start after ~2 transfers
            nc.sync.dma_start(xt[:, 0:4, :], x_r[:, 0:4, :])
            nc.sync.dma_start(wq[:, 0:8, :], w_rs["wqT"][:, 0:8, :])
            nc.sync.dma_start(xt[:, 4:8, :], x_r[:, 4:8, :])
            nc.sync.dma_start(wq[:, 8:16, :], w_rs["wqT"][:, 8:16, :])
            nc.sync.dma_start(xt[:, 8:12, :], x_r[:, 8:12, :])
            nc.sync.dma_start(wk[:, 0:8, :], w_rs["wkT"][:, 0:8, :])
            nc.sync.dma_start(xt[:, 12:16, :], x_r[:, 12:16, :])
            nc.sync.dma_start(wk[:, 8:16, :], w_rs["wkT"][:, 8:16, :])
            nc.sync.dma_start(wv[:, 0:8, :], w_rs["wvT"][:, 0:8, :])
            nc.sync.dma_start(wv[:, 8:16, :], w_rs["wvT"][:, 8:16, :])
            bias_t = pa.tile([128, 4], f32, tag="bias")
            nc.sync.dma_start(bias_t[:], ap["qk_bias"])
            vb_t = pa.tile([1, HDC], f32r, tag="vb")
            nc.sync.dma_start(vb_t[:], ap["v_bias"])
            swap_t = pa.tile([128, 128], f32r, tag="swap")
            nc.sync.dma_start(swap_t[:], ap["swap"])
            for nm in ("cosq", "sinq", "cosk", "sink"):
                nc.sync.dma_start(cw[nm][:], ap[nm])

            # ---- q/k projections, square-sums, split collectives, and the
            # collective-independent part of RoPE (swap + cos/sin combine).
            # Only the final 1/rms multiply waits on the AllGather. ----
            qb = {}
            gth = {}
            for ti, tn in enumerate(("q", "k")):
                wt = wq if tn == "q" else wk
                row_ps = psS.tile([1, SEQ], f32, tag="s", name=f"row_ps_{tn}")
                for h in range(HPC):
                    ps = psS.tile([128, SP], f32, tag="s", name=f"ps_{tn}{h}")
                    for off, n in NCHUNKS:
                        for g in range(KT):
                            nc.tensor.matmul(
                                ps[:, off:off + n],
                                wt[:, g, h * HD:(h + 1) * HD],
                                xt[:, g, off:off + n],
                                start=(g == 0), stop=(g == KT - 1))
                    t_qb = p0.tile([128, SP], f32r, tag=f"{tn}b{h}", name=f"{tn}b{h}")
                    qb[(tn, h)] = t_qb
                    nc.vector.tensor_scalar_add(t_qb[:], ps[:], bias_t[:, 2 * ti + h:2 * ti + h + 1])
                    sq = work.tile([128, SP], f32, tag="sq")
                    nc.vector.tensor_mul(sq[:, 0:SEQ], t_qb[:, 0:SEQ], t_qb[:, 0:SEQ])
                    for off, n in NCHUNKS_REAL:
                        nc.tensor.matmul(row_ps[0:1, off:off + n], ones_col[:],
                                         sq[:, off:off + n],
                                         start=(h == 0), stop=(h == HPC - 1))
                # evict row, launch this tensor's AllGather immediately
                partial_sb = p0.tile([1, SEQ], f32, tag=f"partial{tn}",
                                     name=f"partial{tn}")
                nc.vector.tensor_copy(partial_sb[0:1, :], row_ps[0:1, :])
                bounce_in = dpool.tile([1, SEQ], f32, name=f"bin{tn}")
                bounce_out = dpool.tile([N_CORES, SEQ], f32, name=f"bout{tn}")
                nc.gpsimd.dma_start(bounce_in[:], partial_sb[:])
                nc.gpsimd.collective_compute(
                    "AllGather", mybir.AluOpType.bypass,
                    replica_groups=[list(range(N_CORES))],
                    ins=[bounce_in.opt()], outs=[bounce_out.opt()])
                g_t = p0.tile([N_CORES, SEQ], f32, tag=f"gth{tn}",
                              name=f"gth{tn}")
                nc.gpsimd.dma_start(g_t[:], bounce_out[:])
                gth[tn] = g_t

                if tn == "k":
                    # v projection: fills the PE while the k AllGather is in
                    # flight and lets the pa pool close right after, which
                    # unblocks the attention slab DMAs (pool-close barrier)
                    for st, (s0, m) in enumerate(S_TILES):
                        vp = psS.tile([128, HDC], f32, tag="s")
                        for g in range(KT):
                            nc.tensor.matmul(vp[0:m, :], xt[:, g, s0:s0 + m],
                                             wv[:, g, :],
                                             start=(g == 0), stop=False)
                        nc.tensor.matmul(vp[0:m, :], ones_row_r[0:1, s0:s0 + m],
                                         vb_t[:], start=False, stop=True)
                        nc.vector.tensor_copy(vs[st][0:m, :], vp[0:m, :])

                # collective-independent RoPE: ru = qb*cosW + swap(qb)*sinW,
                # written back in place of qb
                cos_t = cw["cosq" if tn == "q" else "cosk"]
                sin_t = cw["sinq" if tn == "q" else "sink"]
                for h in range(HPC):
                    sw_ps = psS.tile([128, SP], f32, tag="s", name=f"sw_{tn}{h}")
                    for off, n in NCHUNKS:
                        nc.tensor.matmul(sw_ps[:, off:off + n], swap_t[:],
                                         qb[(tn, h)][:, off:off + n],
                                         start=True, stop=True)
                    qbsw = work.tile([128, SP], f32, tag="qbsw")
                    nc.vector.tensor_copy(qbsw[:, 0:SEQ], sw_ps[:, 0:SEQ])
                    t1 = work.tile([128, SP], f32, tag="t1")
                    nc.vector.tensor_mul(t1[:, 0:SEQ], qb[(tn, h)][:, 0:SEQ],
                                         cos_t[:, h * SEQ:h * SEQ + SEQ])
                    nc.vector.tensor_mul(qbsw[:, 0:SEQ], qbsw[:, 0:SEQ],
                                         sin_t[:, h * SEQ:h * SEQ + SEQ])
                    nc.vector.tensor_add(qb[(tn, h)][:, 0:SEQ], t1[:, 0:SEQ],
                                         qbsw[:, 0:SEQ])

            # ---- global mean-square -> rsqrt = exp(-0.5*ln(ms)); both Ln ops
            # then both Exp ops, so the ACT table set switches at most twice ----
            sums = {}
            for ti, tn in enumerate(("q", "k")):
                sums_ps = psS.tile([1, SEQ], f32, tag="s", name=f"sums_ps{ti}")
                for off, n in NCHUNKS_REAL:
                    nc.tensor.matmul(sums_ps[0:1, off:off + n],
                                     ones_col[0:N_CORES, :],
                                     gth[tn][:, off:off + n],
                                     start=True, stop=True)
                ln_t = p0.tile([1, SEQ], f32, tag=f"ln{tn}", name=f"ln{tn}")
                nc.scalar.activation(ln_t[:], sums_ps[0:1, :], AF.Ln,
                                     scale=1.0 / DIM, bias=eps_t[:])
                sums[tn] = ln_t
            for ti, tn in enumerate(("q", "k")):
                nc.scalar.activation(recipf[ti][:], sums[tn][:], AF.Exp,
                                     scale=-0.5)

            # ---- final norm multiply -> rq/rk (f32r), head 0 first so its
            # attention sweep can start earliest ----
            fbt = {}
            for ti, tn in enumerate(("q", "k")):
                fb_ps = psS.tile([128, SEQ], f32, tag="s", name=f"fb_{tn}")
                for off, n in NCHUNKS_REAL:
                    nc.tensor.matmul(fb_ps[:, off:off + n], ones_row[:],
                                     recipf[ti][0:1, off:off + n],
                                     start=True, stop=True)
                fbt[tn] = fb_ps
            for h in range(HPC):
                for tn, out_t in (("q", rq), ("k", rk)):
                    nc.vector.tensor_mul(out_t[h][:, 0:SEQ],
                                         qb[(tn, h)][:, 0:SEQ],
                                         fbt[tn][:, 0:SEQ])



        # ---- attention ----
        with tc.tile_pool(name="epool", bufs=7) as epool, \
             tc.tile_pool(name="att", bufs=2) as att:


            def denom_chain(h, o_ps):
                # denominators -> normalize O^T for head h
                d_ps = psS.tile([1, SEQ], f32, tag="s", name=f"d_ps{h}")
                for off, n in NCHUNKS_REAL:
                    nc.tensor.matmul(d_ps[0:1, off:off + n], ones_col[:],
                                     acc[:, off:off + n], start=True, stop=False)
                    nc.tensor.matmul(d_ps[0:1, off:off + n], ones_col[:],
                                     acc2[:, off:off + n], start=False, stop=True)
                rec_d = att.tile([1, SEQ], f32, tag="rec_d")
                nc.vector.reciprocal(rec_d[:], d_ps[0:1, :])
                fb2 = psS.tile([128, SEQ], f32, tag="s", name=f"fb2_{h}")
                for off, n in NCHUNKS_REAL:
                    nc.tensor.matmul(fb2[:, off:off + n], ones_row[:],
                                     rec_d[0:1, off:off + n], start=True, stop=True)
                fbs = att.tile([128, SEQ], f32, tag="fbs")
                nc.vector.tensor_copy(fbs[:], fb2[:])
                nc.vector.tensor_mul(OT[h][:, 0:SEQ], o_ps[:, 0:SEQ], fbs[:])

            prev = {}
            owt_box = {}
            oproj_box = {}
            for h in range(HPC):
                if h == 1:
                    owt = att.tile([128, HPC, DIM], f32r, tag="owt")
                    owt_box["t"] = owt
                    nc.sync.dma_start(owt[:],
                                      ap["owT"].rearrange("(h p) n -> p h n", p=128))
                o_ps = psO.tile([128, SP], f32, tag="o")
                if False:
                    pass
                pending = []  # software-pipeline: PV/acc of tile t issued after S/exp of t+1

                def s_exp(lhsT_k, m):
                    s_ps = psS.tile([128, SP], f32, tag="s")
                    for off, n in NCHUNKS:
                        nc.tensor.matmul(s_ps[0:m, off:off + n], lhsT_k,
                                         rq[h][:, off:off + n],
                                         start=True, stop=True)
                    e_t = epool.tile([128, SP], f32r, tag="e")
                    nc.scalar.activation(e_t[0:m, 0:SEQ], s_ps[0:m, 0:SEQ],
                                         AF.Exp, scale=SM_SCALE)
                    return e_t

                acc_state = {"n": 0, "first1": True, "first2": True}
                acc_deferred = []  # acc ops held until prev head's denominators read acc

                def acc_op(e_t, m):
                    # denominator accumulation: two independent chains
                    # (DVE 2/3 of tiles, GpSimd 1/3) to keep either off the
                    # critical path
                    i = acc_state["n"]
                    acc_state["n"] += 1
                    if i % 2 == 1:
                        eng, a_t, fkey = nc.gpsimd, acc2, "first2"
                    else:
                        eng, a_t, fkey = nc.vector, acc, "first1"
                    if acc_state[fkey]:
                        acc_state[fkey] = False
                        eng.tensor_copy(a_t[0:m, 0:SEQ], e_t[0:m, 0:SEQ])
                    else:
                        eng.tensor_add(a_t[0:m, 0:SEQ], a_t[0:m, 0:SEQ],
                                       e_t[0:m, 0:SEQ])

                def pv_acc(e_t, lhsT_v, m, first, last):
                    for off, n in NCHUNKS:
                        nc.tensor.matmul(o_ps[:, off:off + n], lhsT_v,
                                         e_t[0:m, off:off + n],
                                         start=first, stop=last)
                    if prev:
                        acc_deferred.append((e_t, m))
                    else:
                        acc_op(e_t, m)

                tcount = {"n": 0}

                def do_tile(lhsT_k, lhsT_v, m, first, last):
                    e_t = s_exp(lhsT_k, m)
                    pending.append((e_t, lhsT_v, m, first, last))
                    if len(pending) > 5:
                        pv_acc(*pending.pop(0))
                    tcount["n"] += 1
                    if tcount["n"] == 4 and prev:
                        denom_chain(prev["h"], prev["o_ps"])
                        prev.clear()
                        for args in acc_deferred:
                            acc_op(*args)
                        acc_deferred.clear()

                # old cache rows first (streamed in 5 slabs per head);
                # new-token tiles last so the v projection can finish while
                # the old-cache sweep runs
                for j in range(NSLAB):
                    ks = att.tile([128, SLAB], f32r, tag="ks")
                    if j == 0:
                        nc.sync.dma_start(ks[:, 0:512],
                                          ap["kTold"][h, :, 0:512])
                        nc.sync.dma_start(ks[:, 512:SLAB],
                                          ap["kTold"][h, :, 512:SLAB])
                    else:
                        nc.sync.dma_start(ks[:], ap["kTold"][h, :, j * SLAB:(j + 1) * SLAB])
                    vsl = att.tile([128, len(SLAB_TILES), HD], f32r, tag="vsl")
                    v_full = ap["vold"][h, j * SLAB:j * SLAB + 2048, :]
                    nc.sync.dma_start(vsl[:, 0:16, :],
                                      v_full.rearrange("(t p) e -> p t e", p=128))
                    nc.sync.dma_start(vsl[0:112, 16, :],
                                      ap["vold"][h, j * SLAB + 2048:(j + 1) * SLAB, :])
                    for t, (t0, m) in enumerate(SLAB_TILES):
                        do_tile(ks[:, t0:t0 + m], vsl[0:m, t, :], m,
                                j == 0 and t == 0, False)
                for st, (s0, m) in enumerate(S_TILES):
                    do_tile(rk[h][:, s0:s0 + m], vs[st][0:m, h * HD:(h + 1) * HD],
                            m, False, st == len(S_TILES) - 1)
                while pending:
                    pv_acc(*pending.pop(0))
                prev.clear()
                prev.update({"h": h, "o_ps": o_ps})

            denom_chain(prev["h"], prev["o_ps"])
            prev.clear()

            # ---- o-projection (per-core partial) ----
            owt = owt_box["t"]
            for s0, m in S_TILES:
                out_sb = att.tile([128, DIM], f32, tag="out_sb")
                for ci in range(DIM // 512):
                    op_ps = psS.tile([128, 512], f32, tag="s")
                    for h in range(HPC):
                        nc.tensor.matmul(op_ps[0:m, :], OT[h][:, s0:s0 + m],
                                         owt[:, h, ci * 512:(ci + 1) * 512],
                                         start=(h == 0), stop=(h == HPC - 1))
                    if ci % 2 == 0:
                        nc.vector.tensor_copy(
                            out_sb[0:m, ci * 512:(ci + 1) * 512], op_ps[0:m, :])
                    else:
                        nc.scalar.copy(
                            out_sb[0:m, ci * 512:(ci + 1) * 512], op_ps[0:m, :])
                nc.sync.dma_start(ap["out"][s0:s0 + m, :], out_sb[0:m, :])


def _patch_act_tables(nc):
    """All ACT funcs used here (Exp, Ln, Copy) live in act-func-set 6
    (natural_log_exp_and_others); the auto-inserted per-function set loads
    thrash between exp/ln sets at ~1.3us per switch. Retarget every load to
    set 6 and drop redundant ones."""
    for blk in nc.main_func.blocks:
        keep = []
        seen = False
        for ins in blk.instructions:
            if isinstance(ins, mybir.InstLoadActFuncSet):
                ins.act_func_set_id = 6
                si = ins.sync_info
                clean = si is None or (len(si.on_wait) == 0 and len(si.on_update) == 0)
                if seen and clean:
                    continue  # redundant reload of the same set
                seen = True
            keep.append(ins)
        blk.instructions[:] = keep


def _dram_tensors(nc):
    d = {}
    d["xT"] = nc.dram_tensor("xT", [DIM, SP], f32r, kind="ExternalInput")
    d["wqT"] = nc.dram_tensor("wqT", [DIM, HDC], f32r, kind="ExternalInput")
    d["wkT"] = nc.dram_tensor("wkT", [DIM, HDC], f32r, kind="ExternalInput")
    d["wvT"] = nc.dram_tensor("wvT", [DIM, HDC], f32r, kind="ExternalInput")
    d["owT"] = nc.dram_tensor("owT", [HDC, DIM], f32r, kind="ExternalInput")
    d["qk_bias"] = nc.dram_tensor("qk_bias", [128, 4], f32, kind="ExternalInput")
    d["v_bias"] = nc.dram_tensor("v_bias", [1, HDC], f32r, kind="ExternalInput")
    d["swap"] = nc.dram_tensor("swap", [128, 128], f32r, kind="ExternalInput")
    d["ones_r"] = nc.dram_tensor("ones_r", [1, SP], f32r, kind="ExternalInput")
    d["cosq"] = nc.dram_tensor("cosq", [128, HPC * SEQ], f32, kind="ExternalInput")
    d["sinq"] = nc.dram_tensor("sinq", [128, HPC * SEQ], f32, kind="ExternalInput")
    d["cosk"] = nc.dram_tensor("cosk", [128, HPC * SEQ], f32, kind="ExternalInput")
    d["sink"] = nc.dram_tensor("sink", [128, HPC * SEQ], f32, kind="ExternalInput")
    d["kTold"] = nc.dram_tensor("kTold", [HPC, 128, OLD], f32r, kind="ExternalInput")
    d["vold"] = nc.dram_tensor("vold", [HPC, OLD, HD], f32r, kind="ExternalInput")
    d["out"] = nc.dram_tensor("out", [SEQ, DIM], f32, kind="ExternalOutput")
    return d


def _build():
    nc = bacc.Bacc("TRN2", target_bir_lowering=False, debug=False,
                   num_devices=N_CORES)
    d = _dram_tensors(nc)
    with tile.TileContext(nc) as tc:
        _emit(nc, tc, d)
    nc.compile()
    _patch_act_tables(nc)
    return nc


_NC_CACHE = None


def _get_nc():
    global _NC_CACHE
    if _NC_CACHE is None:
        _NC_CACHE = _build()
    return _NC_CACHE


def _prep_inputs(x, q_w, q_b, k_w, k_b, v_w, v_b, o_w, o_b, norm_q_w, norm_k_w,
                 cache_k, cache_v, freqs_cos, freqs_sin,
                 current_start, frame_seqlen, sink_tokens):
    cs, sink = int(current_start), int(sink_tokens)
    rolling = CACHE - sink
    local_start = (cs - sink) % rolling + sink
    old_idx = np.r_[0:local_start, local_start + SEQ:CACHE]
    assert old_idx.size == OLD

    xT = np.zeros((DIM, SP), dtype=np.float32)
    xT[:, 0:SEQ] = np.ascontiguousarray(x[0].T)

    # RoPE/norm tables in T layout: cos_full[d, s] = cos[s, d//2] * w[d];
    # sin_full[d, s] = sin[s, d//2] * w[d^1] * (-1 if d even else +1)
    dd = np.arange(HD)
    cos_d = freqs_cos.T[dd // 2, :]            # [128, 720]
    sin_d = freqs_sin.T[dd // 2, :]
    sign = np.where(dd % 2 == 0, -1.0, 1.0).astype(np.float32)[:, None]
    swap_m = np.zeros((HD, HD), dtype=np.float32)
    swap_m[dd, dd ^ 1] = 1.0

    ck = np.asarray(cache_k[0])                # [11520, 16, 128]
    cv = np.asarray(cache_v[0])
    ck_old = ck[old_idx]                       # [10800, 16, 128]
    cv_old = cv[old_idx]

    in_maps = []
    for c in range(N_CORES):
        hs = slice(c * HDC, (c + 1) * HDC)
        heads = [c * HPC + h for h in range(HPC)]
        bias4 = np.zeros((128, 4), dtype=np.float32)
        for h in range(HPC):
            bias4[:, 0 + h] = q_b[hs][h * HD:(h + 1) * HD]
            bias4[:, 2 + h] = k_b[hs][h * HD:(h + 1) * HD]
        cosq = np.empty((128, HPC * SEQ), dtype=np.float32)
        sinq = np.empty((128, HPC * SEQ), dtype=np.float32)
        cosk = np.empty((128, HPC * SEQ), dtype=np.float32)
        sink_t = np.empty((128, HPC * SEQ), dtype=np.float32)
        for h in range(HPC):
            wqn = np.asarray(norm_q_w)[hs][h * HD:(h + 1) * HD]
            wkn = np.asarray(norm_k_w)[hs][h * HD:(h + 1) * HD]
            sl = slice(h * SEQ, (h + 1) * SEQ)
            cosq[:, sl] = cos_d * wqn[:, None]
            sinq[:, sl] = sin_d * wqn[dd ^ 1][:, None] * sign
            cosk[:, sl] = cos_d * wkn[:, None]
            sink_t[:, sl] = sin_d * wkn[dd ^ 1][:, None] * sign
        kT_old = np.ascontiguousarray(
            ck_old[:, heads, :].transpose(1, 2, 0))          # [2, 128, 10800]
        v_old = np.ascontiguousarray(
            cv_old[:, heads, :].transpose(1, 0, 2))          # [2, 10800, 128]
        in_maps.append({
            "xT": xT,
            "wqT": np.ascontiguousarray(q_w[hs, :].T),
            "wkT": np.ascontiguousarray(k_w[hs, :].T),
            "wvT": np.ascontiguousarray(v_w[hs, :].T),
            "owT": np.ascontiguousarray(o_w[:, hs].T),
            "qk_bias": bias4,
            "v_bias": np.asarray(v_b[hs], dtype=np.float32).reshape(1, HDC),
            "swap": swap_m,
            "ones_r": np.ones((1, SP), dtype=np.float32),
            "cosq": cosq, "sinq": sinq, "cosk": cosk, "sink": sink_t,
            "kTold": kT_old,
            "vold": v_old,
        })
    return in_maps


def run_spmd(in_maps, **kw):
    nc = _get_nc()
    return bass_utils.run_bass_kernel_spmd(
        nc, in_maps, core_ids=list(range(N_CORES)), **kw)


def kernel(**inputs):
    inputs = {k: np.asarray(v) if not np.isscalar(v) else v
              for k, v in inputs.items()}
    in_maps = _prep_inputs(**inputs)
    res = run_spmd(in_maps)
    out = np.zeros((SEQ, DIM), dtype=np.float32)
    for c in range(N_CORES):
        out += res.results[c]["out"]
    out += np.asarray(inputs["o_b"], dtype=np.float32)[None, :]
    return out[None].astype(np.float32)

